# revision 53
# baseline (speedup 1.0000x reference)
"""GAT (2-layer, 4-head) kernel for 8 Trainium2 NeuronCores — v3.

Design (vs the v2 baseline):
  - Table rows are 256B (128 fp16 cols, 80 used) END-TO-END: the encoder
    writes t1_full with 256B rows, layer-1 output is staged at 256B rows,
    and the inter-layer AllGather ships 256B rows and lands DIRECTLY as
    the layer-2 gather table — the 12.8MB strided "expansion" DMA of v2
    (~780us of serialized DMA in sim) disappears entirely.
  - Global table row order is (core, slot, window): row = r*shard_pad +
    p*w_pc + w.  With TSPLIT = 5*shard_pad the A/B int16 gather halves
    align exactly with source cores 0-4 vs 5-7, so A/B membership is
    known before balancing and the 2D window balance applies to every
    shard (no boundary-shard special case).  Slot-major rows also make
    the encoder's table write ONE bulk DMA per shard (contiguous 12.5KB
    runs per partition instead of 160B rows).
  - ONE AllGather ships the whole staged layer-1 output (t2_shard,
    n-major == the table layout) into Shared t2_full: a single writer
    (the Tile framework requires exactly one writer instruction per
    Shared DRAM tensor) and the best-bandwidth regime of the collective.
  - One-hot S matrices built on DVE in fp16 ([p,c,j] vs iota), S^T via PE
    transposes (fp16 PSUM) copied out on Act/DVE; z = asrc+adst accumulated
    on PE; w = exp(leaky_relu(z)) = max(exp(z), exp(0.2 z)).
  - Attention-source logits stored DUPLICATED (x2) in table rows so the
    message multiply is a single fp16 2x DVE op per gather region.
  - A/B halves gather on separate SWDGE queues, rotating across all 4
    queues per super (A on si%4, B on (si+2)%4), with triple-buffered
    gather tiles — the descriptor streams and DMA rings of consecutive
    supers overlap (paired-bench ~30% combined win vs one queue).
  - _bench_ns measures the AMORTIZED per-execution time: the axon-PJRT
    tunnel has a fixed ~80ms dispatch round-trip latency independent of
    the kernel (a trivial 2-DMA kernel measures the same single-dispatch
    wall), so K executions are issued asynchronously (they pipeline on
    the device) and the slope between a small-K and big-K batch is
    reported — the fixed latency cancels exactly.
"""

import math
import os

import numpy as np

H = 4
P = 128
EPS = 1e-16
NEG_SLOPE = 0.2
RW = 128          # table row width (fp16) -> 256B gather granule
UC = 80           # used cols: 64 feat + 8 asrc-dup + 8 adst-dup
SUP = 6           # windows per gather super-batch == rows chunk
ENC_CHUNK = 512

LAST_RESULTS = None
LAST_BENCH_NS = None


def _f32(x):
    return np.ascontiguousarray(x, dtype=np.float32)


def _f16(x):
    return np.ascontiguousarray(np.asarray(x, dtype=np.float32), dtype=np.float16)


def _balance(wA, wB, n_win, cap_nodes=P):
    """Greedy 2D balance of nodes into n_win windows of <=cap_nodes."""
    n = len(wA)
    order = np.argsort(-(wA + wB))
    loadA = np.zeros(n_win)
    loadB = np.zeros(n_win)
    cnt = np.zeros(n_win, np.int64)
    assign = np.empty(n, np.int64)
    mA = max(wA.sum() / n_win, 1.0)
    mB = max(wB.sum() / n_win, 1.0)
    for i in order:
        score = np.maximum((loadA + wA[i]) / mA, (loadB + wB[i]) / mB)
        score[cnt >= cap_nodes] = np.inf
        j = int(np.argmin(score))
        assign[i] = j
        loadA[j] += wA[i]
        loadB[j] += wB[i]
        cnt[j] += 1
    return assign


def _wrap16(v, cap):
    out = np.zeros((16, cap // 16), np.int16)
    out[np.arange(v.size) % 16, np.arange(v.size) // 16] = v
    return np.tile(out, (8, 1))


def _prep_host(inputs):
    x = _f32(inputs["x"])
    ei = np.asarray(inputs["edge_index"])
    N = x.shape[0]
    NC = 8
    shard = N // NC
    w_pc = math.ceil(shard / P)
    shard_pad = w_pc * P
    rows_total = NC * shard_pad

    # A/B gather halves must be int16-addressable; with slot-major rows
    # (row = r*shard_pad + p*w_pc + w) a core-aligned split keeps A/B
    # membership independent of the window assignment.
    tsplit = 5 * shard_pad
    assert 0 < tsplit <= 32768 and rows_total - tsplit <= 32768

    loop = np.arange(N, dtype=np.int64)
    src = np.concatenate([ei[0].astype(np.int64), loop])
    dst = np.concatenate([ei[1].astype(np.int64), loop])
    srcA = src < 5 * shard        # source owned by cores 0-4 -> A half

    # ---- window assignment: 2D balance (A-half/B-half in-counts) ----
    pos = np.tile(np.arange(shard), NC).reshape(NC, shard)
    for r in range(NC):
        sel = (dst >= r * shard) & (dst < (r + 1) * shard)
        dl = dst[sel] - r * shard
        eA = srcA[sel]
        wA = np.bincount(dl[eA], minlength=shard).astype(np.float64)
        wB = np.bincount(dl[~eA], minlength=shard).astype(np.float64)
        asg = _balance(wA, wB, w_pc)
        for w in range(w_pc):
            m = np.where(asg == w)[0]
            pos[r][m] = w * P + np.arange(len(m))

    def trow_of(nodes):
        r = nodes // shard
        pp = pos[r, nodes % shard]
        return r * shard_pad + (pp % P) * w_pc + pp // P

    trow = trow_of(src)

    # ---- per-core edge slots ----
    per_core = []
    max_a = max_b = 0
    for r in range(NC):
        sel = (dst >= r * shard) & (dst < (r + 1) * shard)
        tr = trow[sel]
        dslot = pos[r, dst[sel] - r * shard]       # position within shard
        wins = []
        for w in range(w_pc):
            m = (dslot >= w * P) & (dslot < (w + 1) * P)
            trw, dlw = tr[m], dslot[m] - w * P
            a = trw < tsplit
            wins.append((trw[a], dlw[a], trw[~a] - tsplit, dlw[~a]))
            max_a = max(max_a, int(a.sum()))
            max_b = max(max_b, int((~a).sum()))
        per_core.append(wins)

    t_a = max(P, math.ceil(max_a / P) * P)
    t_b = max(P, math.ceil(max_b / P) * P)
    sl_a, sl_b = t_a // P, t_b // P
    s_w = sl_a + sl_b

    cores = []
    for r in range(NC):
        idx_a = np.zeros((P, w_pc * (t_a // 16)), np.int16)
        idx_b = np.zeros((P, w_pc * (t_b // 16)), np.int16)
        dwrap = np.full((P, w_pc * s_w), -1.0, np.float16)
        for w, (ta, da, tb, db) in enumerate(per_core[r]):
            idx_a[:, w * (t_a // 16):(w + 1) * (t_a // 16)] = _wrap16(ta, t_a)
            idx_b[:, w * (t_b // 16):(w + 1) * (t_b // 16)] = _wrap16(tb, t_b)
            dwrap[np.arange(ta.size) % P,
                  w * s_w + np.arange(ta.size) // P] = da
            dwrap[np.arange(tb.size) % P,
                  w * s_w + sl_a + np.arange(tb.size) // P] = db
        xo = np.zeros((7, shard_pad), np.float16)
        xo[:, pos[r]] = x[r * shard:(r + 1) * shard].T.astype(np.float16)
        cores.append({"idxA": idx_a, "idxB": idx_b, "dwrap": dwrap, "xT_own": xo})

    # ---- shared inputs ----
    # xT columns stay in COMPUTE order (core, window, slot) — the encoder
    # reads contiguous slabs; only the t1/t2 table row order is chunk-major.
    xT_full = np.zeros((7, rows_total), np.float16)
    xT_full[:, (np.arange(N) // shard) * shard_pad
            + pos[np.arange(N) // shard, np.arange(N) % shard]] = \
        x.T.astype(np.float16)

    iota_mat = np.broadcast_to(
        np.arange(P, dtype=np.float32)[None, :, None], (P, P, s_w)
    ).reshape(P, P * s_w).astype(np.float16).copy()

    # ---- weights ----
    g1w = _f32(inputs["g1_w"])                       # [64, 256]
    a1s, a1d = _f32(inputs["g1_as"]), _f32(inputs["g1_ad"])
    g2w = _f32(inputs["g2_w"])                       # [256, 64]
    a2s, a2d = _f32(inputs["g2_as"]), _f32(inputs["g2_ad"])
    c1 = g1w.shape[1] // H
    c2 = g2w.shape[1] // H

    atil_s = np.stack([g1w[:, h * c1:(h + 1) * c1] @ a1s[h] for h in range(H)], 1)
    atil_d = np.stack([g1w[:, h * c1:(h + 1) * c1] @ a1d[h] for h in range(H)], 1)
    atil_sd_dup = np.concatenate(
        [np.repeat(atil_s, 2, 1), np.repeat(atil_d, 2, 1)], 1)   # [64, 16]

    enc_w2 = _f32(inputs["enc_w2"])
    enc_b2 = _f32(inputs["enc_b2"])
    # zero-pad the projections to full 256B table rows (cols 80:128 are
    # dead but must be defined bytes for the bulk row DMAs)
    W2p = np.concatenate([enc_w2, enc_w2 @ atil_sd_dup,
                          np.zeros((64, RW - UC), np.float32)], 1)  # [64,128]
    brow = np.concatenate([enc_b2, enc_b2 @ atil_sd_dup,
                           np.zeros(RW - UC, np.float32)])          # [128]
    biasrow = np.broadcast_to(brow, (P, RW)).astype(np.float16).copy()
    W2ad = enc_w2 @ np.repeat(atil_d, 2, 1)                       # [64, 8]
    badrow = np.broadcast_to(enc_b2 @ np.repeat(atil_d, 2, 1),
                             (P, 2 * H)).astype(np.float16).copy()

    m2s = np.zeros((H * c2, H), np.float32)
    m2d = np.zeros((H * c2, H), np.float32)
    for h in range(H):
        m2s[h * c2:(h + 1) * c2, h] = a2s[h]
        m2d[h * c2:(h + 1) * c2, h] = a2d[h]
    m2sd_dup = np.concatenate([np.repeat(m2s, 2, 1), np.repeat(m2d, 2, 1)], 1)
    W2p2 = np.concatenate([g2w, g2w @ m2sd_dup,
                           np.zeros((2 * P, RW - UC), np.float32)], 1)
    # post1 computes hp = ELU(o1)+1; subtract the column sums to correct
    bias2row = np.broadcast_to(-W2p2.sum(0), (P, RW)).astype(np.float16).copy()
    W2p2 = np.concatenate([W2p2[0:P], W2p2[P:2 * P]], 1)          # [128, 256]

    b1 = _f32(inputs["g1_b"])

    consts = {
        "xT": xT_full,
        "iota_mat": iota_mat,
        "identity": np.eye(P, dtype=np.float16),
        "ident4f": np.eye(H, dtype=np.float32),
        "epsrow": np.full((P, 2 * H), 1e-5, np.float16),
        "enc_w1": _f16(inputs["enc_w1"]),
        "b1c": _f32(inputs["enc_b1"])[:, None],
        "W2p": _f16(W2p),
        "biasrow": biasrow,
        "W2ad": _f16(W2ad),
        "badrow": badrow,
        "g1w": _f16(g1w),
        "b1cols": b1.reshape(2, P).T.copy(),      # [128, 2] fp32
        "W2p2": _f16(W2p2),
        "bias2row": bias2row,
        "g2bc": _f32(inputs["g2_b"])[:, None],
        "dw1": _f16(inputs["dec_w1"]),
        "db1c": _f32(inputs["dec_b1"])[:, None],
        # g2 bias folded through dec_w1 (o2Ts becomes a plain copy)
        "db1cp": _f32(inputs["dec_b1"]
                      + _f32(inputs["dec_w1"]).T @ _f32(inputs["g2_b"]))[:, None],
        "dw2": _f16(inputs["dec_w2"]),
        "db2c": _f32(inputs["dec_b2"])[:, None],
    }

    geom = dict(N=N, NC=NC, shard=shard, w_pc=w_pc, shard_pad=shard_pad,
                rows_total=rows_total, t_a=t_a, t_b=t_b, sl_a=sl_a,
                sl_b=sl_b, s_w=s_w, c1=c1, c2=c2, tsplit=tsplit)
    return geom, cores, consts, pos


def _build(geom):
    import concourse.bass as bass  # noqa: F401
    import concourse.mybir as mybir
    import concourse.tile as tile
    from concourse import bacc

    dt = mybir.dt
    AF = mybir.ActivationFunctionType
    OP = mybir.AluOpType

    NC, w_pc, shard_pad = geom["NC"], geom["w_pc"], geom["shard_pad"]
    rows_total = geom["rows_total"]
    t_a, t_b, sl_a, sl_b, s_w = (geom["t_a"], geom["t_b"], geom["sl_a"],
                                 geom["sl_b"], geom["s_w"])
    c2 = geom["c2"]
    tsplit = geom["tsplit"]
    rg = [list(range(NC))]

    nc = bacc.Bacc("TRN2", target_bir_lowering=False, debug=False,
                   enable_asserts=False, num_devices=NC,
                   num_swdge_queues=4)

    xT = nc.dram_tensor("xT", [7, rows_total], dt.float16,
                        kind="ExternalInput").ap()
    xT_own = nc.dram_tensor("xT_own", [7, shard_pad], dt.float16,
                            kind="ExternalInput").ap()
    idxA = nc.dram_tensor("idxA", [P, w_pc * (t_a // 16)], dt.int16,
                          kind="ExternalInput").ap()
    idxB = nc.dram_tensor("idxB", [P, w_pc * (t_b // 16)], dt.int16,
                          kind="ExternalInput").ap()
    dwrap = nc.dram_tensor("dwrap", [P, w_pc * s_w], dt.float16,
                           kind="ExternalInput").ap()
    cns = {}
    for name, shape, d in [
        ("iota_mat", [P, P * s_w], dt.float16),
        ("identity", [P, P], dt.float16),
        ("ident4f", [H, H], dt.float32),
        ("epsrow", [P, 2 * H], dt.float16),
        ("enc_w1", [7, 64], dt.float16),
        ("b1c", [64, 1], dt.float32),
        ("W2p", [64, RW], dt.float16),
        ("biasrow", [P, RW], dt.float16),
        ("W2ad", [64, 2 * H], dt.float16),
        ("badrow", [P, 2 * H], dt.float16),
        ("g1w", [64, 2 * P], dt.float16),
        ("b1cols", [P, 2], dt.float32),
        ("W2p2", [P, 2 * RW], dt.float16),
        ("bias2row", [P, RW], dt.float16),
        ("g2bc", [64, 1], dt.float32),
        ("dw1", [64, 64], dt.float16),
        ("db1c", [64, 1], dt.float32),
        ("db1cp", [64, 1], dt.float32),
        ("dw2", [64, H], dt.float16),
        ("db2c", [H, 1], dt.float32),
    ]:
        cns[name] = nc.dram_tensor(name, shape, d, kind="ExternalInput").ap()
    out = nc.dram_tensor("out", [shard_pad, H], dt.float32,
                         kind="ExternalOutput").ap()

    supers = [list(range(s, min(s + SUP, w_pc))) for s in range(0, w_pc, SUP)]

    with tile.TileContext(nc) as tc:
        with tc.tile_pool(name="dram", bufs=1, space="DRAM") as dram:
            t1_full = dram.tile([rows_total, RW], dt.float16)
            t2_shard = dram.tile([shard_pad, RW], dt.float16)
            # AllGather lands each chunk DIRECTLY as table rows (chunk-major
            # global row order); Shared so collectives may write it.
            t2_full = dram.tile([rows_total, RW], dt.float16,
                                addr_space="Shared", name="t2_full")

            with tc.tile_pool(name="cpool", bufs=1) as cpool:
                csb = {}
                for name, ap in cns.items():
                    t = cpool.tile(ap.shape, ap.dtype, name=f"c_{name}",
                                   tag=f"c_{name}")
                    nc.sync.dma_start(t[:], ap)
                    csb[name] = t
                idxA_sb = cpool.tile(idxA.shape, dt.int16, tag="idxA_sb")
                nc.sync.dma_start(idxA_sb[:], idxA)
                idxB_sb = cpool.tile(idxB.shape, dt.int16, tag="idxB_sb")
                nc.sync.dma_start(idxB_sb[:], idxB)
                dwrap_sb = cpool.tile(dwrap.shape, dt.float16, tag="dwrap_sb")
                nc.sync.dma_start(dwrap_sb[:], dwrap)
                adst1_sb = cpool.tile([P, 2 * H * w_pc], dt.float16,
                                      tag="adst1_sb")
                adst2_sb = cpool.tile([P, 2 * H * w_pc], dt.float16,
                                      tag="adst2_sb")
                ident = csb["identity"]

                # ================= encoder (replicated) =================
                # Table rows are 256B, slot-major (row = p*w_pc + w within
                # a shard), so each partition writes one contiguous
                # w_pc*256B run — ONE bulk DMA per shard.
                with nc.named_scope("enc"), \
                     tc.tile_pool(name="encs", bufs=2) as encs, \
                     tc.tile_pool(name="encp", bufs=3, space="PSUM") as encp:
                    for r in range(NC):
                        off = r * shard_pad
                        if r % 2 == 0:
                            xsl = encs.tile([7, 2 * shard_pad], dt.float16,
                                            tag="xsl")
                            nc.scalar.dma_start(
                                xsl[:], xT[:, off:off + 2 * shard_pad])
                        pk = encs.tile([P, w_pc * RW], dt.float16, tag="pk")
                        x0 = (r % 2) * shard_pad
                        c0 = 0
                        while c0 < shard_pad:
                            cw = min(ENC_CHUNK, shard_pad - c0)
                            nb = cw // P
                            ps1 = encp.tile([64, ENC_CHUNK], dt.float32,
                                            tag="ps1")
                            nc.tensor.matmul(ps1[:, :cw], csb["enc_w1"][:],
                                             xsl[:, x0 + c0:x0 + c0 + cw],
                                             start=True, stop=True)
                            h1a = encs.tile([64, ENC_CHUNK], dt.float16,
                                            tag="h1a")
                            hw2 = cw // 2
                            nc.scalar.activation(h1a[:, :hw2], ps1[:, :hw2],
                                                 AF.Relu, bias=csb["b1c"][:])
                            nc.vector.tensor_scalar(
                                out=h1a[:, hw2:cw], in0=ps1[:, hw2:cw],
                                scalar1=csb["b1c"][:], scalar2=0.0,
                                op0=OP.add, op1=OP.max)
                            pb = encp.tile([P, (ENC_CHUNK // P) * RW],
                                           dt.float32, tag="pb")
                            for b in range(nb):
                                nc.tensor.matmul(pb[:, b * RW:(b + 1) * RW],
                                                 h1a[:, b * P:(b + 1) * P],
                                                 csb["W2p"][:],
                                                 start=True, stop=True)
                            bb = c0 // P
                            nc.vector.tensor_tensor(
                                out=pk[:, bb * RW:(bb + nb) * RW].rearrange(
                                    "p (b c) -> p b c", c=RW),
                                in0=pb[:, :nb * RW].rearrange(
                                    "p (b c) -> p b c", b=nb),
                                in1=csb["biasrow"][:].unsqueeze(1)
                                    .to_broadcast([P, nb, RW]),
                                op=OP.add)
                            c0 += cw
                        nc.sync.dma_start(
                            t1_full[off:off + shard_pad, :].rearrange(
                                "(p b) c -> p b c", b=w_pc),
                            pk[:].rearrange("p (b c) -> p b c", c=RW))

                # ===== mini-encoder: adst1 for own shard =====
                with nc.named_scope("minienc"), \
                     tc.tile_pool(name="mes", bufs=2) as mes, \
                     tc.tile_pool(name="mep", bufs=2, space="PSUM") as mep:
                    xo = mes.tile([7, shard_pad], dt.float16, tag="xc2",
                                  bufs=1)
                    nc.scalar.dma_start(xo[:], xT_own[:])
                    c0 = 0
                    while c0 < shard_pad:
                        cw = min(ENC_CHUNK, shard_pad - c0)
                        nb = cw // P
                        ps1 = mep.tile([64, ENC_CHUNK], dt.float32, tag="mps1")
                        nc.tensor.matmul(ps1[:, :cw], csb["enc_w1"][:],
                                         xo[:, c0:c0 + cw], start=True,
                                         stop=True)
                        h1a = mes.tile([64, ENC_CHUNK], dt.float16, tag="mh1a")
                        nc.scalar.activation(h1a[:, :cw], ps1[:, :cw], AF.Relu,
                                             bias=csb["b1c"][:])
                        pb = mep.tile([P, (ENC_CHUNK // P) * 2 * H], dt.float32,
                                      tag="mpb")
                        for b in range(nb):
                            nc.tensor.matmul(pb[:, b * 2 * H:(b + 1) * 2 * H],
                                             h1a[:, b * P:(b + 1) * P],
                                             csb["W2ad"][:],
                                             start=True, stop=True)
                        w0 = c0 // P
                        nc.vector.tensor_tensor(
                            out=adst1_sb[:, w0 * 2 * H:(w0 + nb) * 2 * H]
                                .rearrange("p (b c) -> p b c", b=nb),
                            in0=pb[:, :nb * 2 * H].rearrange(
                                "p (b c) -> p b c", b=nb),
                            in1=csb["badrow"][:].unsqueeze(1)
                                .to_broadcast([P, nb, 2 * H]),
                            op=OP.add)
                        c0 += cw

                # ================= GAT layers =================
                ablate_coll = bool(os.environ.get("GAT_ABLATE_COLL"))

                def gat_layer(layer):
                    ch = 64 if layer == 1 else c2
                    mw = H * ch                       # message width
                    fw = mw + 2 * H                   # + dup denominator
                    tfull = t1_full if layer == 1 or ablate_coll else t2_full
                    adst_sb = adst1_sb if layer == 1 else adst2_sb

                    with nc.named_scope(f"gat{layer}"), \
                         tc.tile_pool(name=f"gs{layer}", bufs=2) as gs, \
                         tc.tile_pool(name=f"gp{layer}", bufs=2,
                                      space="PSUM") as gp, \
                         tc.tile_pool(name=f"gp1{layer}", bufs=1,
                                      space="PSUM") as gp1:
                        for si, sup in enumerate(supers):
                            nw = len(sup)
                            w0 = sup[0]
                            qa, qb = si % 4, (si + 2) % 4
                            hgb3 = 3
                            hgA = gs.tile([P, SUP * sl_a * P], dt.float16,
                                          tag="hgA", bufs=hgb3)
                            nc.gpsimd.dma_gather(
                                out_ap=hgA[:, :nw * sl_a * P].rearrange(
                                    "p (s c) -> p s c", c=P),
                                in_ap=tfull[0:tsplit, :],
                                idxs_ap=idxA_sb[:, w0 * (t_a // 16):
                                                (w0 + nw) * (t_a // 16)],
                                num_idxs=nw * t_a, num_idxs_reg=nw * t_a,
                                elem_size=P, single_packet=False,
                                queue_num=qa)
                            hgB = gs.tile([P, SUP * sl_b * P], dt.float16,
                                          tag="hgB", bufs=hgb3)
                            nc.gpsimd.dma_gather(
                                out_ap=hgB[:, :nw * sl_b * P].rearrange(
                                    "p (s c) -> p s c", c=P),
                                in_ap=tfull[tsplit:rows_total, :],
                                idxs_ap=idxB_sb[:, w0 * (t_b // 16):
                                                (w0 + nw) * (t_b // 16)],
                                num_idxs=nw * t_b, num_idxs_reg=nw * t_b,
                                elem_size=P, single_packet=False,
                                queue_num=qb)
                            if layer == 1:
                                pk2 = gs.tile([P, SUP * RW], dt.float16,
                                              tag="pk2", bufs=2)
                            else:
                                opk = gs.tile([P, SUP * H], dt.float32,
                                              tag="opk", bufs=2)

                            for wl, w in enumerate(sup):
                                _window(nc, tc, csb, gs, gp, gp1, dwrap_sb,
                                        adst_sb, hgA, hgB, pk2 if layer == 1
                                        else opk, adst2_sb, layer, ch, mw, fw,
                                        sl_a, sl_b, s_w, w, wl, ident, dt, AF,
                                        OP)

                            if layer == 1:
                                nc.sync.dma_start(
                                    t2_shard[:].rearrange(
                                        "(p b) c -> p b c", b=w_pc)
                                    [:, w0:w0 + nw, :],
                                    pk2[:, :nw * RW].rearrange(
                                        "p (b c) -> p b c", c=RW))
                            else:
                                nc.sync.dma_start(
                                    out[P * w0:P * (w0 + nw), :].rearrange(
                                        "(b p) c -> p b c", p=P),
                                    opk[:, :nw * H].rearrange(
                                        "p (b c) -> p b c", b=nw))

                        if layer == 1 and not ablate_coll:
                            # one AllGather: staged rows land directly as
                            # the (n-major, slot-major) layer-2 table
                            with nc.named_scope("ag2"):
                                nc.gpsimd.collective_compute(
                                    "AllGather", mybir.AluOpType.bypass,
                                    replica_groups=rg,
                                    ins=[t2_shard[:].rearrange(
                                        "r c -> (r c)").opt()],
                                    outs=[t2_full[:].rearrange(
                                        "r c -> (r c)").opt()])

                gat_layer(1)
                gat_layer(2)

    nc.compile()
    return nc


def _window(nc, tc, csb, gs, gp, gp1, dwrap_sb, adst_sb, hgA, hgB, outtile,
            adst2_sb, layer, ch, mw, fw, sl_a, sl_b, s_w, w, wl, ident, dt,
            AF, OP):
    """One 128-dst window of a GAT layer."""
    # ---- S one-hot [p, c, j] (fp16, 2x) ----
    sall = gs.tile([P, P * s_w], dt.float16, tag="sall",
                   bufs=4 if layer == 1 else 6)
    sv = sall[:].rearrange("p (c j) -> p c j", j=s_w)
    iv = csb["iota_mat"][:].rearrange("p (c j) -> p c j", j=s_w)
    nc.vector.tensor_tensor(
        out=sv[:],
        in0=dwrap_sb[:, w * s_w:(w + 1) * s_w].unsqueeze(1)
            .to_broadcast([P, P, s_w]),
        in1=iv[:],
        op=OP.is_equal)

    # ---- S^T via PE transpose; copies split across Act/Pool/DVE ----
    st = gs.tile([P, s_w * P], dt.float16, tag="st",
                 bufs=4 if layer == 1 else 6)
    ng = (s_w + 6) // 7
    for g in range(ng):
        j0, j1 = g * 7, min((g + 1) * 7, s_w)
        stp = gp1.tile([P, 7 * P], dt.float16, tag="stp",
                       bufs=2 if layer == 1 else 3)
        if os.environ.get("GAT_ABLATE_TRANS"):   # timing probe only
            nc.tensor.transpose(stp[:, 0:P],
                                sall[:].rearrange("p (c j) -> p c j",
                                                  j=s_w)[:, :, j0],
                                ident[:])
        else:
            for j in range(j0, j1):
                nc.tensor.transpose(
                    stp[:, (j - j0) * P:(j - j0 + 1) * P],
                    sall[:].rearrange("p (c j) -> p c j", j=s_w)[:, :, j],
                    ident[:])
        if g == ng - 1:
            nc.vector.tensor_copy(st[:, j0 * P:j1 * P],
                                  stp[:, :(j1 - j0) * P])
        else:
            nc.scalar.activation(st[:, j0 * P:j1 * P],
                                 stp[:, :(j1 - j0) * P], AF.Identity)

    # ---- z = asrc + adst-gather on PE ----
    # (a merged multi-slab asrc matmul via a 3D moving AP miscomputes on
    # HW — rel err 0.021 vs 0.0085 — so keep per-slab matmuls)
    zps = gp.tile([P, s_w * 2 * H], dt.float32, tag="zps", bufs=1)
    if not os.environ.get("GAT_ZMERGE"):
        for j in range(s_w):
            nc.tensor.matmul(zps[:, j * 2 * H:(j + 1) * 2 * H],
                             st[:, j * P:(j + 1) * P],
                             adst_sb[:, w * 2 * H:(w + 1) * 2 * H],
                             start=True, stop=False)
            hgX = hgA if j < sl_a else hgB
            jj = (wl * sl_a + j) if j < sl_a else (wl * sl_b + j - sl_a)
            nc.tensor.matmul(zps[:, j * 2 * H:(j + 1) * 2 * H],
                             ident[:],
                             hgX[:].rearrange("p (s c) -> p s c", c=P)
                             [:, jj, 64:64 + 2 * H],
                             start=False, stop=True)
    else:
        for (hgX, lo, n_s) in ((hgA, 0, sl_a), (hgB, sl_a, sl_b)):
            nc.tensor.matmul(
                zps[:, lo * 2 * H:(lo + n_s) * 2 * H].rearrange(
                    "p (s c) -> p s c", c=2 * H),
                ident[:],
                hgX[:].rearrange("p (s c) -> p s c", c=P)
                [:, wl * n_s:(wl + 1) * n_s, 64:64 + 2 * H],
                start=True, stop=False, skip_group_check=True)
        for j in range(s_w):
            nc.tensor.matmul(zps[:, j * 2 * H:(j + 1) * 2 * H],
                             st[:, j * P:(j + 1) * P],
                             adst_sb[:, w * 2 * H:(w + 1) * 2 * H],
                             start=False, stop=(j == s_w - 1),
                             skip_group_check=True)

    # ---- w = exp(lrelu(z)) = max(exp(z), exp(0.2 z)); w lands in the
    # per-slab tail of msg so numerator+denominator aggregate in ONE
    # matmul per slab (sall_j stationary loaded once, not twice) ----
    e1 = gs.tile([P, s_w * 2 * H], dt.float16, tag="e1")
    nc.scalar.activation(e1[:], zps[:], AF.Exp)
    e2 = gs.tile([P, s_w * 2 * H], dt.float16, tag="e2")
    nc.scalar.activation(e2[:], zps[:], AF.Exp, scale=NEG_SLOPE)
    msg = gs.tile([P, s_w * fw], dt.float16, tag="msg", bufs=3)
    msgv = msg[:].rearrange("p (j c) -> p j c", c=fw)
    # w into a contiguous tile (the multiply's broadcast AP needs the
    # j-stride == h-extent coalescing to stay <=3 free dims) AND into the
    # per-slab tail of msg for the fused num+den aggregation
    wexp = gs.tile([P, s_w * 2 * H], dt.float16, tag="wexp", bufs=3)
    nc.vector.tensor_tensor(out=wexp[:], in0=e1[:], in1=e2[:], op=OP.max)
    nc.vector.tensor_tensor(
        out=msgv[:, :, mw:fw],
        in0=e1[:].rearrange("p (j c) -> p j c", c=2 * H),
        in1=e2[:].rearrange("p (j c) -> p j c", c=2 * H),
        op=OP.max)

    # ---- messages: msg[p, j, h, c] = feat * w (fp16 2x) ----
    for (hgX, lo, n_s) in ((hgA, 0, sl_a), (hgB, sl_a, sl_b)):
        base = (wl * n_s) * P
        hgv = hgX[:, base:base + n_s * P].rearrange("p (s c) -> p s c", c=P)
        if layer == 1:
            in0 = (hgv[:, :, 0:ch]
                   .rearrange("p s (c d) -> p s c d", d=2)
                   .unsqueeze(2).to_broadcast([P, n_s, H, ch // 2, 2]))
        else:
            in0 = (hgv[:, :, 0:mw]
                   .rearrange("p s (h c d) -> p s h c d", h=H, c=ch // 2,
                              d=2))
        nc.vector.tensor_tensor(
            out=msgv[:, lo:lo + n_s, 0:mw].rearrange(
                "p j (h c d) -> p j h c d", h=H, d=2),
            in0=in0,
            in1=wexp[:, lo * 2 * H:(lo + n_s) * 2 * H].rearrange(
                "p (j h d) -> p j h d", h=H, d=2)
                .unsqueeze(3).to_broadcast([P, n_s, H, ch // 2, 2]),
            op=OP.mult)

    # ---- aggregate on PE (numerator + denominator fused per slab) ----
    agg = gp.tile([P, fw], dt.float32, tag="agg")
    for j in range(s_w):
        nc.tensor.matmul(agg[:, 0:fw],
                         sall[:].rearrange("p (c j) -> p c j", j=s_w)[:, :, j],
                         msg[:, j * fw:(j + 1) * fw],
                         start=(j == 0), stop=False, skip_group_check=True)
    nc.tensor.matmul(agg[:, mw:fw], ident[:], csb["epsrow"][:],
                     start=False, stop=True, skip_group_check=True)

    # ---- softmax normalize (L1 on Pool — DVE busier; L2 keeps Pool free
    # for the gathers that trail the collective) ----
    rec = gs.tile([P, 2 * H], dt.float32, tag="rec")
    nc.vector.reciprocal(rec[:], agg[:, mw:fw])
    asb = gs.tile([P, mw], dt.float16, tag="asb")
    nc.vector.tensor_tensor(
        out=asb[:].rearrange("p (h c d) -> p h c d", h=H, d=2),
        in0=agg[:, 0:mw].rearrange("p (h c d) -> p h c d", h=H, d=2),
        in1=rec[:].rearrange("p (h d) -> p h d", d=2)
            .unsqueeze(2).to_broadcast([P, H, ch // 2, 2]),
        op=OP.mult)

    if layer == 1:
        _post1(nc, csb, gs, gp1, asb, outtile, adst2_sb, wl, w, ident, dt,
               AF, OP)
    else:
        _post2(nc, csb, gs, gp1, asb, outtile, wl, ident, dt, AF, OP)


def _post1(nc, csb, gs, gp1, asb, pk2, adst2_sb, wl, w, ident, dt, AF, OP):
    """Project heads, ELU(+1), L2 projection + logits, pack table row."""
    # aggT[64, h*128] = asb^T per head
    t4 = gp1.tile([64, H * P], dt.float16, tag="t4")
    for h in range(H):
        nc.tensor.transpose(t4[:, P * h:P * (h + 1)],
                            asb[:, 64 * h:64 * (h + 1)], ident[:])
    aggT = gs.tile([64, H * P], dt.float16, tag="aggT")
    nc.scalar.activation(aggT[:], t4[:], AF.Identity)
    # o1 feat-major [c, dst] + o2 share one PSUM bank
    o1 = gp1.tile([P, 2 * P + RW], dt.float32, tag="o1", bufs=2)
    for h in range(H):
        nc.tensor.matmul(o1[64 * (h % 2):64 * (h % 2) + 64,
                            P * (h // 2):P * (h // 2) + P],
                         csb["g1w"][:, 64 * h:64 * (h + 1)],
                         aggT[:, P * h:P * (h + 1)], start=True, stop=True,
                         skip_group_check=True)
    hps = []
    for t in range(2):
        ps = o1[:, P * t:P * (t + 1)]
        ex = gs.tile([P, P], dt.float16, tag=f"ex{t}")
        nc.scalar.activation(ex[:], ps, AF.Exp,
                             bias=csb["b1cols"][:, t:t + 1])
        rl = gs.tile([P, P], dt.float16, tag=f"rl{t}")
        nc.scalar.activation(rl[:], ps, AF.Relu,
                             bias=csb["b1cols"][:, t:t + 1])
        hp = gs.tile([P, P], dt.float16, tag=f"hp{t}")
        nc.vector.scalar_tensor_tensor(out=hp[:], in0=ex[:], scalar=1.0,
                                       in1=rl[:], op0=OP.min, op1=OP.add)
        hps.append(hp)
    # o2[dst, 128] = hp^T @ W2p2 (stationary = hp halves)
    o2 = o1[:, 2 * P:2 * P + RW]
    nc.tensor.matmul(o2, hps[0][:], csb["W2p2"][:, 0:RW],
                     start=True, stop=False, skip_group_check=True)
    nc.tensor.matmul(o2, hps[1][:], csb["W2p2"][:, RW:2 * RW],
                     start=False, stop=True, skip_group_check=True)
    nc.vector.tensor_tensor(out=pk2[:, wl * RW:(wl + 1) * RW], in0=o2,
                            in1=csb["bias2row"][:], op=OP.add)
    nc.vector.tensor_copy(adst2_sb[:, w * 2 * H:(w + 1) * 2 * H],
                          pk2[:, wl * RW + 72:wl * RW + UC])


def _post2(nc, csb, gs, gp1, asb, opk, wl, ident, dt, AF, OP):
    """decoder MLP on DVE tensor_scalar ops (g2 bias folded into db1c'),
    write node-major output."""
    o2T = gp1.tile([64, P], dt.float16, tag="o2T")
    nc.tensor.transpose(o2T[:], asb[:], ident[:])
    o2Ts = gs.tile([64, P], dt.float16, tag="o2Ts")
    nc.scalar.activation(o2Ts[:], o2T[:], AF.Identity, bias=csb["g2bc"][:])
    dec = gp1.tile([P, P + H], dt.float32, tag="dec")
    d1 = dec[0:64, 0:P]
    nc.tensor.matmul(d1, csb["dw1"][:], o2Ts[:], start=True, stop=True)
    d1s = gs.tile([64, P], dt.float16, tag="d1s")
    nc.vector.tensor_scalar(out=d1s[:], in0=d1, scalar1=csb["db1c"][:],
                            scalar2=0.0, op0=OP.add, op1=OP.max)
    d2 = dec[64:64 + H, 0:P]
    nc.tensor.matmul(d2, csb["dw2"][:], d1s[:], start=True, stop=True)
    d2s = gs.tile([H, P], dt.float32, tag="d2s")
    nc.scalar.activation(d2s[:], d2, AF.Identity, bias=csb["db2c"][:])
    oT = dec[:, P:P + H]
    nc.tensor.transpose(oT, d2s[:], csb["ident4f"][:])
    nc.vector.tensor_copy(opk[:, H * wl:H * (wl + 1)], oT)


def _bench_ns(nc, in_maps, n_cores, k_small=8, k_big=88, reps=7):
    """Amortized per-NEFF-execution time via pipelined dispatch.

    The axon-PJRT tunnel has a fixed ~80ms dispatch round-trip latency
    that is independent of the kernel (a trivial 2-DMA kernel measures
    the same single-dispatch wall as the full GAT kernel).  Executions
    issued asynchronously pipeline on the device, so the slope between
    a small-K and a big-K batch isolates the true marginal time per
    execution; the fixed latency cancels exactly.
    """
    import time as _time

    import jax
    import numpy as _np
    from jax.experimental.shard_map import shard_map
    from jax.sharding import Mesh, NamedSharding, PartitionSpec

    import concourse.mybir as mybir
    from concourse import bass2jax

    bass2jax.install_neuronx_cc_hook()
    partition_name = (nc.partition_id_tensor.name
                      if nc.partition_id_tensor else None)
    in_names, out_names, out_avals, zero_outs = [], [], [], []
    for alloc in nc.m.functions[0].allocations:
        if not isinstance(alloc, mybir.MemoryLocationSet):
            continue
        name = alloc.memorylocations[0].name
        if alloc.kind == "ExternalInput":
            if name != partition_name:
                in_names.append(name)
        elif alloc.kind == "ExternalOutput":
            out_names.append(name)
            shape = tuple(alloc.tensor_shape)
            dtype = mybir.dt.np(alloc.dtype)
            out_avals.append(jax.core.ShapedArray(shape, dtype))
            zero_outs.append(_np.zeros(shape, dtype))
    n_params = len(in_names)
    n_outs = len(out_names)
    all_names = tuple(in_names + out_names +
                      ([partition_name] if partition_name else []))

    def _fn(*args):
        ins = args[:n_params]
        zouts = tuple(args[n_params:])
        operands = list(ins) + list(zouts)
        if partition_name:
            operands.append(bass2jax.partition_id_tensor())
        outs = bass2jax._bass_exec_p.bind(
            *operands, out_avals=tuple(out_avals), in_names=all_names,
            out_names=tuple(out_names), lowering_input_output_aliases=(),
            sim_require_finite=True, sim_require_nnan=True, nc=nc)
        return tuple(outs)

    devices = jax.devices()[:n_cores]
    mesh = Mesh(_np.asarray(devices), ("core",))
    spec = PartitionSpec("core")
    sh = NamedSharding(mesh, spec)
    in_specs = (spec,) * (n_params + n_outs)
    out_specs = (spec,) * n_outs
    concat_in = [
        jax.device_put(_np.concatenate(
            [_np.asarray(in_maps[c][nm]) for c in range(n_cores)], axis=0), sh)
        for nm in in_names]
    fn = jax.jit(shard_map(_fn, mesh=mesh,
                           in_specs=in_specs, out_specs=out_specs,
                           check_rep=False), keep_unused=True)
    zsets = [[jax.device_put(
        _np.zeros((n_cores * z.shape[0], *z.shape[1:]), z.dtype), sh)
        for z in zero_outs] for _ in range(k_big)]

    def wall(K):
        t0 = _time.perf_counter()
        outs = [fn(*concat_in, *zsets[k]) for k in range(K)]
        jax.block_until_ready(outs)
        return _time.perf_counter() - t0

    wall(2)  # warmup + compile
    slopes = []
    for _ in range(reps):
        ws = wall(k_small)
        wb = wall(k_big)
        slopes.append((wb - ws) / (k_big - k_small))
    slopes.sort()
    per = slopes[len(slopes) // 2]
    return per * 1e9, slopes[0] * 1e9


def kernel(**inputs):
    global LAST_RESULTS, LAST_BENCH_NS
    from concourse.bass_utils import run_bass_kernel_spmd
    from concourse.bass_interp import get_hw_module

    geom, cores, consts, pos = _prep_host(inputs)
    nc = _build(geom)
    nc.m = get_hw_module(nc.m)

    in_maps = []
    for core in cores:
        m = dict(core)
        m.update(consts)
        in_maps.append(m)

    res = run_bass_kernel_spmd(nc, in_maps, core_ids=list(range(geom["NC"])),
                               trace=bool(os.environ.get("GAT_TRACE")))
    LAST_RESULTS = res

    if os.environ.get("GAT_BENCH"):
        LAST_BENCH_NS = _bench_ns(nc, in_maps, geom["NC"])

    N, shard, shard_pad = geom["N"], geom["shard"], geom["shard_pad"]
    out = np.empty((N, H), np.float32)
    for r in range(geom["NC"]):
        out[r * shard:(r + 1) * shard] = res.results[r]["out"][pos[r]]
    return out
